# revision 3
# baseline (speedup 1.0000x reference)
"""GRU stack kernel for TRN2 — development module.

Layouts (per core, Bc=4):
  h_all  [128, 24] fp16   col = j*8 + c*4 + b     (j=layer, c=u-chunk, b=batch)
  XIN0   [128, TW*4] fp16 col = t*4 + b           (partitions = d)
  XTOP   [128, 2*TA*4] fp16  col = c*(TA*4) + s*4 + b
  weights lhsT fp16:
    wu   [128, 3*1536]  col = j*1536 + ku*768 + g*128 + m
    ww   [128, 2*1536]  (layers 1, 2)
    weff [128, 1536]
    w0   [128, 768]
    wd   [128, 256]     col = ku*128 + n   (rhs for final dense)
"""
import numpy as np
import concourse.bass as bass
import concourse.tile as tile
from concourse import bacc, mybir
from contextlib import ExitStack

FP16 = mybir.dt.float16
FP32 = mybir.dt.float32
AF = mybir.ActivationFunctionType

B, D, U = 4, 128, 256  # per-core batch
NG = 6   # gate chunks of 128 (3U = 768)


def prep_consts(inp):
    """Host-side constant prep (numpy). inp: full-input dict (fp32 numpy)."""
    def to16(x):
        return np.ascontiguousarray(np.asarray(x, np.float32).astype(np.float16))

    Ws = [inp['W0'], inp['W1'], inp['W2']]
    Us = [inp['U0'], inp['U1'], inp['U2']]
    bs = [inp['b0'], inp['b1'], inp['b2']]
    Wd, bd = inp['Wd'], inp['bd']
    Weff = Wd.astype(np.float32) @ Ws[0].astype(np.float32)
    beff = bs[0][0] + bd.astype(np.float32) @ Ws[0].astype(np.float32)

    def lhsT_tiles(Wmat):  # [256, 768] -> [128, 1536]
        return Wmat.reshape(2, 128, NG, 128).transpose(1, 0, 2, 3).reshape(128, 2 * NG * 128)

    wu = np.concatenate([lhsT_tiles(Uj) for Uj in Us], axis=1)          # [128, 4608]
    ww = np.concatenate([lhsT_tiles(Wj) for Wj in Ws[1:]], axis=1)      # [128, 3072]
    weff = lhsT_tiles(Weff)                                             # [128, 1536]
    w0 = Ws[0]                                                          # [128, 768]
    wd = Wd.reshape(2, 128, 128).transpose(1, 0, 2).reshape(128, 256)   # [128, 256]

    bzr_sets = [(bs[j][0] + bs[j][1])[:512].reshape(4, 128) for j in range(3)]
    bzr_sets.append((beff + bs[0][1])[:512].reshape(4, 128))
    bzr = np.concatenate(bzr_sets, axis=1)                              # [4, 512]
    bn1 = np.concatenate([bs[j][1][512:].reshape(2, 128) for j in range(3)], axis=1)
    bn2_sets = [bs[j][0][512:].reshape(2, 128) for j in range(3)]
    bn2_sets.append(beff[512:].reshape(2, 128))
    bn2 = np.concatenate(bn2_sets, axis=1)                              # [2, 512]

    mzr = np.zeros((4, 16), np.float32)
    for c in range(4):
        mzr[c, c * 4:(c + 1) * 4] = 1.0
    mn = np.zeros((2, 8), np.float32)
    for c in range(2):
        mn[c, c * 4:(c + 1) * 4] = 1.0
    ones1 = np.ones((1, 128), np.float32)
    bdr = bd.reshape(1, 128)

    # merged per-cell bias: one k=8 matmul covers zr(4 chunks), n1(2), n2(2)
    # bball[k, set*128+m]; mball[k, f] selects the 4-col block f//4 == k
    bball_sets = []
    for s in range(4):
        j = 0 if s == 3 else s
        zr_part = bzr_sets[s]                       # [4, 128]
        n1_part = bs[j][1][512:].reshape(2, 128)
        n2_part = bn2_sets[s]
        bball_sets.append(np.concatenate([zr_part, n1_part, n2_part], axis=0))
    bball = np.concatenate(bball_sets, axis=1)      # [8, 512]
    mball = np.zeros((8, 32), np.float32)
    for k in range(8):
        mball[k, k * 4:(k + 1) * 4] = 1.0

    h0 = np.zeros((128, 24), np.float32)
    for j in range(3):
        hj = np.asarray(inp['h_init'][j], np.float32).reshape(2, 128)
        for c in range(2):
            for b in range(4):
                h0[:, j * 8 + c * 4 + b] = hj[c]

    return {k: to16(v) for k, v in dict(
        wu=wu, ww=ww, weff=weff, w0=w0, wd=wd, bzr=bzr, bn1=bn1, bn2=bn2,
        mzr=mzr, mn=mn, ones1=ones1, bdr=bdr, h0=h0,
        bball=bball, mball=mball).items()}


CONST_SHAPES = dict(wu=[128, 4608], ww=[128, 3072], weff=[128, 1536], w0=[128, 768],
                    wd=[128, 256], bzr=[4, 512], bn1=[2, 384], bn2=[2, 512],
                    mzr=[4, 16], mn=[2, 8], ones1=[1, 128], bdr=[1, 128], h0=[128, 24],
                    bball=[8, 512], mball=[8, 32])


def build_nc(TW=256, TA=256, n_reps=1, num_devices=8, unroll=4, wave=True,
             gates_off=False):
    """TW warmup steps, TA total output steps (AR steps = TA-1)."""
    TT4 = TA * 4
    nc = bacc.Bacc("TRN2", target_bir_lowering=False, debug=False,
                   num_devices=num_devices)

    xin = nc.dram_tensor("xin", [B, TW, D], FP32, kind="ExternalInput")
    nz = nc.dram_tensor("nz", [B, TW, D], FP32, kind="ExternalInput")
    out = nc.dram_tensor("out", [B, TA, D], FP32, kind="ExternalOutput")
    cts = {k: nc.dram_tensor(k, v, FP16, kind="ExternalInput")
           for k, v in CONST_SHAPES.items()}

    with tile.TileContext(nc) as tc, ExitStack() as ctx:
        const = ctx.enter_context(tc.tile_pool(name="const", bufs=1))
        state = ctx.enter_context(tc.tile_pool(name="state", bufs=1))
        ct = {}
        for k, shp in CONST_SHAPES.items():
            ct[k] = const.tile(shp, FP16, tag=k, name=f"c_{k}")
            nc.sync.dma_start(ct[k][:], cts[k].ap())

        h_all = state.tile([128, 24], FP16, tag="h_all")
        # b-major columns: xin0 col = b*TW + t ; xtop col = (c*4 + b)*TA + s
        xin0 = state.tile([128, TW * 4], FP16, tag="xin0")
        xtop = state.tile([128, 2 * TT4], FP16, tag="xtop")
        # warmup wavefront staging: XW[s] = layer-(s+1) input stream,
        # col = s*(8*TW) + (c*4+b)*TW + t
        xw = state.tile([128, 2 * 8 * TW], FP16, tag="xw")

        # ---- preamble: inputs+noise -> fp16, transposed into xin0 (b-major) ----
        xin_f = xin.ap().rearrange("b t d -> (b t) d")
        nz_f = nz.ap().rearrange("b t d -> (b t) d")
        ntile = (TW * 4 + 127) // 128
        with tc.tile_pool(name="pre", bufs=3) as pre:
            for i in range(ntile):
                rows = min(128, TW * 4 - i * 128)
                xt = pre.tile([128, 128], FP32, tag="xt")
                nt = pre.tile([128, 128], FP32, tag="nt")
                nc.sync.dma_start(xt[:rows, :], xin_f[i * 128:i * 128 + rows, :])
                nc.sync.dma_start(nt[:rows, :], nz_f[i * 128:i * 128 + rows, :])
                xs = pre.tile([128, 128], FP16, tag="xs")
                nc.vector.tensor_add(xs[:rows, :], xt[:rows, :], nt[:rows, :])
                nc.sync.dma_start_transpose(xin0[:, i * 128:i * 128 + rows], xs[:rows, :])

        loop_ctx = ExitStack()
        psum = loop_ctx.enter_context(tc.tile_pool(name="psum", bufs=2, space="PSUM"))
        gp = loop_ctx.enter_context(tc.tile_pool(name="gates", bufs=6))

        def cell_early(j, mode):
            """Bias + recurrent (gh) matmuls for layer j — depend only on h_all[j].
            One PSUM bank per cell: [0:16]=zr, [16:24]=n1(hn'), [24:32]=n2(xn')."""
            bset = 3 if (j == 0 and mode == 'ar') else j
            pp = psum.tile([128, 32], FP32, tag=f"pp{j}", name=f"pp{j}")
            nc.tensor.matmul(pp[:], ct['bball'][0:8, bset * 128:(bset + 1) * 128],
                             ct['mball'][0:8, :], start=True, stop=False)
            for g in range(4):
                for ku in range(2):
                    nc.tensor.matmul(pp[:, g * 4:(g + 1) * 4],
                                     ct['wu'][:, j * 1536 + ku * 768 + g * 128:
                                              j * 1536 + ku * 768 + (g + 1) * 128],
                                     h_all[:, j * 8 + ku * 4:j * 8 + ku * 4 + 4],
                                     start=False, stop=False)
            for g in range(4, NG):
                for ku in range(2):
                    nc.tensor.matmul(pp[:, (g - 4) * 4 + 16:(g - 3) * 4 + 16],
                                     ct['wu'][:, j * 1536 + ku * 768 + g * 128:
                                              j * 1536 + ku * 768 + (g + 1) * 128],
                                     h_all[:, j * 8 + ku * 4:j * 8 + ku * 4 + 4],
                                     start=False, stop=False)
            return pp

        def emit_gx(j, pp, mode, tj):
            """Input-side matmuls for layer j consuming its input at time tj
            (zr chunks first). In warm mode, layer inputs come from xin0/XW."""
            if j == 0 and mode in ('warm', 'warm_serial'):
                kx = 1
                wx = lambda ku, g: ct['w0'][:, g * 128:(g + 1) * 128]
                rhs = lambda ku: xin0_v[:, :, bass.ds(tj, 1)]
            elif j == 0:
                kx = 2
                wx = lambda ku, g: ct['weff'][:, ku * 768 + g * 128:
                                              ku * 768 + (g + 1) * 128]
                rhs = lambda ku, _t=tj: xtop_v[:, ku * 4:(ku + 1) * 4, bass.ds(_t - 1, 1)]
            elif mode == 'warm':
                off = (j - 1) * 1536
                kx = 2
                wx = lambda ku, g: ct['ww'][:, off + ku * 768 + g * 128:
                                            off + ku * 768 + (g + 1) * 128]
                rhs = lambda ku, _t=tj: xw_v[:, (j - 1) * 8 + ku * 4:
                                             (j - 1) * 8 + (ku + 1) * 4, bass.ds(_t, 1)]
            else:
                off = (j - 1) * 1536
                kx = 2
                wx = lambda ku, g: ct['ww'][:, off + ku * 768 + g * 128:
                                            off + ku * 768 + (g + 1) * 128]
                if j == 1:
                    rhs = lambda ku: h_all[:, ku * 4:ku * 4 + 4]
                else:
                    x2 = x2_ref[0]
                    rhs = lambda ku: x2[:, ku * 4:ku * 4 + 4]
            for g in range(4):
                for ku in range(kx):
                    nc.tensor.matmul(pp[:, g * 4:(g + 1) * 4], wx(ku, g), rhs(ku),
                                     start=False, stop=False)
            for g in range(4, NG):
                for ku in range(kx):
                    nc.tensor.matmul(pp[:, (g - 4) * 4 + 24:(g - 3) * 4 + 24],
                                     wx(ku, g), rhs(ku),
                                     start=False, stop=(g == NG - 1 and ku == kx - 1))

        def r31(ap8):
            return ap8.rearrange("p (cb one) -> p cb one", one=1)

        def emit_gates(j, pp, mode, tj):
            if gates_off:
                # timing diagnostic: keep the h dependency, skip the real chain
                nc.vector.tensor_copy(h_all[:, j * 8:j * 8 + 8], pp[:, 0:8])
                if mode != 'warm' and j == 1:
                    x2 = gp.tile([128, 8], FP16, tag="x2", name="x2")
                    x2_ref[0] = x2
                    nc.vector.tensor_copy(x2[:], h_all[:, 8:16])
                return
            zr_sb = gp.tile([128, 16], FP32, tag="zr_sb", name="zr_sb")
            nc.scalar.activation(zr_sb[:], pp[:, 0:16], AF.Sigmoid)
            t1 = gp.tile([128, 8], FP32, tag="t1", name="t1")
            nc.vector.tensor_mul(t1[:], zr_sb[:, 8:16], pp[:, 16:24])
            t2 = gp.tile([128, 8], FP32, tag="t2", name="t2")
            nc.vector.tensor_add(t2[:], t1[:], pp[:, 24:32])
            n_sb = gp.tile([128, 8], FP32, tag="n_sb", name="n_sb")
            nc.scalar.activation(n_sb[:], t2[:], AF.Tanh)
            dd = gp.tile([128, 8], FP32, tag="dd", name="dd")
            nc.vector.tensor_sub(dd[:], h_all[:, j * 8:j * 8 + 8], n_sb[:])
            ee = gp.tile([128, 8], FP32, tag="ee", name="ee")
            nc.vector.tensor_mul(ee[:], zr_sb[:, 0:8], dd[:])
            nc.vector.tensor_add(h_all[:, j * 8:j * 8 + 8], n_sb[:], ee[:])
            if mode == 'warm':
                if j == 0:
                    nc.scalar.activation(xw_v[:, 0:8, bass.ds(tj, 1)],
                                         r31(h_all[:, 0:8]), AF.Copy)
                elif j == 1:
                    nc.vector.tensor_add(xw_v[:, 8:16, bass.ds(tj, 1)],
                                         r31(h_all[:, 8:16]),
                                         xw_v[:, 0:8, bass.ds(tj, 1)])
                elif top_write[0]:
                    nc.vector.tensor_add(xtop_v[:, :, 0:1], r31(h_all[:, 16:24]),
                                         xw_v[:, 8:16, bass.ds(tj, 1)])
            else:
                if j == 1:
                    x2 = gp.tile([128, 8], FP16, tag="x2", name="x2")
                    x2_ref[0] = x2
                    nc.vector.tensor_add(x2[:], h_all[:, 8:16], h_all[:, 0:8])
                elif j == 2:
                    x2 = x2_ref[0]
                    top = (xtop_v[:, :, 0:1] if mode == 'warm_serial'
                           else xtop_v[:, :, bass.ds(tj, 1)])
                    nc.vector.tensor_add(top, r31(h_all[:, 16:24]), r31(x2[:]))

        # views: xin0_v [128, 4b, TW]; xtop_v [128, 8(c,b), TA]; xw_v [128, 16, TW]
        xin0_v = xin0[:].rearrange("p (b t) -> p b t", b=4)
        xtop_v = xtop[:].rearrange("p (cb t) -> p cb t", t=TA)
        xw_v = xw[:].rearrange("p (s t) -> p s t", t=TW)
        x2_ref = [None]
        top_write = [True]

        def warm_cell(j, tj):
            pp = cell_early(j, 'warm')
            emit_gx(j, pp, 'warm', tj)
            emit_gates(j, pp, 'warm', tj)

        def emit_gates_staged(cells):
            """Stage-ordered gate math for independent wavefront cells so the
            three chains interleave on each engine queue. cells: [(j, pp, tj)]."""
            zs, t1s, t2s, ns, ds_, es = [], [], [], [], [], []
            for (j, pp, tj) in cells:
                z = gp.tile([128, 16], FP32, tag="zr_sb", name="zr_sb")
                nc.scalar.activation(z[:], pp[:, 0:16], AF.Sigmoid)
                zs.append(z)
            for (j, pp, tj), z in zip(cells, zs):
                t1 = gp.tile([128, 8], FP32, tag="t1", name="t1")
                nc.vector.tensor_mul(t1[:], z[:, 8:16], pp[:, 16:24])
                t1s.append(t1)
            for (j, pp, tj), t1 in zip(cells, t1s):
                t2 = gp.tile([128, 8], FP32, tag="t2", name="t2")
                nc.vector.tensor_add(t2[:], t1[:], pp[:, 24:32])
                t2s.append(t2)
            for (j, pp, tj), t2 in zip(cells, t2s):
                n_sb = gp.tile([128, 8], FP32, tag="n_sb", name="n_sb")
                nc.scalar.activation(n_sb[:], t2[:], AF.Tanh)
                ns.append(n_sb)
            for (j, pp, tj), n_sb in zip(cells, ns):
                dd = gp.tile([128, 8], FP32, tag="dd", name="dd")
                nc.vector.tensor_sub(dd[:], h_all[:, j * 8:j * 8 + 8], n_sb[:])
                ds_.append(dd)
            for (j, pp, tj), z, dd in zip(cells, zs, ds_):
                ee = gp.tile([128, 8], FP32, tag="ee", name="ee")
                nc.vector.tensor_mul(ee[:], z[:, 0:8], dd[:])
                es.append(ee)
            for (j, pp, tj), n_sb, ee in zip(cells, ns, es):
                nc.vector.tensor_add(h_all[:, j * 8:j * 8 + 8], n_sb[:], ee[:])
            for (j, pp, tj) in cells:
                if j == 0:
                    nc.scalar.activation(xw_v[:, 0:8, bass.ds(tj, 1)],
                                         r31(h_all[:, 0:8]), AF.Copy)
                elif j == 1:
                    nc.vector.tensor_add(xw_v[:, 8:16, bass.ds(tj, 1)],
                                         r31(h_all[:, 8:16]),
                                         xw_v[:, 0:8, bass.ds(tj, 1)])

        def wave_body(iv0, n):
            # wavefront: cells (0,iv), (1,iv-1), (2,iv-2) are independent.
            pend = [cell_early(j, 'warm') for j in range(3)]
            for i in range(n):
                iv = iv0 + i
                for j in range(3):
                    emit_gx(j, pend[j], 'warm', iv - j)
                emit_gates_staged([(j, pend[j], iv - j) for j in range(3)])
                if i < n - 1:
                    pend = [cell_early(j, 'warm') for j in range(3)]

        def ar_body(iv0, n):
            pend = [cell_early(j, 'ar') for j in range(3)]
            for i in range(n):
                iv = iv0 + i
                for j in range(3):
                    emit_gx(j, pend[j], 'ar', iv)
                    cur = pend[j]
                    emit_gates(j, cur, 'ar', iv)
                    if i < n - 1:
                        pend[j] = cell_early(j, 'ar')

        def warm_serial_body(iv0, n):
            pend = [cell_early(j, 'warm') for j in range(3)]
            for i in range(n):
                iv = iv0 + i
                for j in range(3):
                    emit_gx(j, pend[j], 'warm_serial', iv)
                    cur = pend[j]
                    emit_gates(j, cur, 'warm_serial', iv)
                    if i < n - 1:
                        pend[j] = cell_early(j, 'warm')

        def one_pass():
            nc.vector.tensor_copy(h_all[:], ct['h0'][:])
            if wave:
                # warmup wavefront: prologue ticks 0..1, steady 2..TW-1, epilogue
                warm_cell(0, 0)
                warm_cell(0, 1)
                warm_cell(1, 0)
                tc.For_i_unrolled_general(2, TW, 1, wave_body, max_unroll=unroll)
                warm_cell(1, TW - 1)
                warm_cell(2, TW - 2)
                warm_cell(2, TW - 1)
            else:
                tc.For_i_unrolled_general(0, TW, 1, warm_serial_body, max_unroll=unroll)
            tc.For_i_unrolled_general(1, TA, 1, ar_body, max_unroll=unroll)

        if n_reps == 1:
            one_pass()
        else:
            with tc.For_i(0, n_reps, 1):
                one_pass()

        loop_ctx.close()

        # ---- final dense: preds[t] = XTOP[t] @ Wd + bd  (b-major rows) ----
        out_f = out.ap().rearrange("b t d -> (b t) d")
        with tc.tile_pool(name="dpsum", bufs=2, space="PSUM") as dpsum, \
             tc.tile_pool(name="dout", bufs=3) as dout:
            ndt = (TA * 4 + 127) // 128
            for i in range(ndt):
                rows = min(128, TA * 4 - i * 128)
                pd = dpsum.tile([128, 128], FP32, tag="pd")
                nc.tensor.matmul(pd[:rows, :], ct['ones1'][0:1, 0:rows], ct['bdr'][0:1, :],
                                 start=True, stop=False)
                for ku in range(2):
                    nc.tensor.matmul(pd[:rows, :],
                                     xtop[:, ku * TT4 + i * 128: ku * TT4 + i * 128 + rows],
                                     ct['wd'][:, ku * 128:(ku + 1) * 128],
                                     start=False, stop=(ku == 1))
                ob = dout.tile([128, 128], FP32, tag="ob")
                nc.scalar.activation(ob[:rows, :], pd[:rows, :], AF.Copy)
                nc.sync.dma_start(out_f[i * 128:i * 128 + rows, :], ob[:rows, :])

    nc.compile()
    return nc




# ---------------- SPMD runner + public kernel() entry point ----------------
N_CORES = 8
_CACHE = {}


def _make_runner(nc, n_cores=N_CORES):
    import jax
    from jax.sharding import Mesh, PartitionSpec
    from jax.experimental.shard_map import shard_map
    from concourse.bass2jax import _bass_exec_p, install_neuronx_cc_hook, \
        partition_id_tensor

    install_neuronx_cc_hook()
    partition_name = nc.partition_id_tensor.name if nc.partition_id_tensor else None
    in_names, out_names, out_avals, zero_outs = [], [], [], []
    for alloc in nc.m.functions[0].allocations:
        if not isinstance(alloc, mybir.MemoryLocationSet):
            continue
        name = alloc.memorylocations[0].name
        if alloc.kind == "ExternalInput":
            if name != partition_name:
                in_names.append(name)
        elif alloc.kind == "ExternalOutput":
            out_names.append(name)
            shape = tuple(alloc.tensor_shape)
            dtype = mybir.dt.np(alloc.dtype)
            out_avals.append(jax.core.ShapedArray(shape, dtype))
            zero_outs.append(np.zeros(shape, dtype))
    n_params = len(in_names)
    all_in_names = list(in_names) + list(out_names)
    if partition_name is not None:
        all_in_names.append(partition_name)

    def _body(*args):
        operands = list(args)
        if partition_name is not None:
            operands.append(partition_id_tensor())
        outs = _bass_exec_p.bind(
            *operands, out_avals=tuple(out_avals), in_names=tuple(all_in_names),
            out_names=tuple(out_names), lowering_input_output_aliases=(),
            sim_require_finite=True, sim_require_nnan=True, nc=nc)
        return tuple(outs)

    devices = jax.devices()[:n_cores]
    mesh = Mesh(np.asarray(devices), ("core",))
    in_specs = (PartitionSpec("core"),) * (n_params + len(out_avals))
    out_specs = (PartitionSpec("core"),) * len(out_names)
    sharded = jax.jit(shard_map(_body, mesh=mesh, in_specs=in_specs,
                                out_specs=out_specs, check_rep=False),
                      keep_unused=True)

    def run(in_maps, time_only=False):
        import jax
        per_core = [[np.asarray(m[name]) for name in in_names] for m in in_maps]
        concat_in = [np.concatenate([per_core[c][i] for c in range(n_cores)], axis=0)
                     for i in range(n_params)]
        concat_zeros = [np.zeros((n_cores * z.shape[0], *z.shape[1:]), z.dtype)
                        for z in zero_outs]
        o = sharded(*concat_in, *concat_zeros)
        jax.block_until_ready(o)
        if time_only:
            return None
        return [{name: np.asarray(o[i]).reshape(n_cores, *out_avals[i].shape)[c]
                 for i, name in enumerate(out_names)} for c in range(n_cores)]
    return run


def _get_runner(n_reps=1):
    key = ('runner', n_reps)
    if key not in _CACHE:
        nc = build_nc(TW=256, TA=256, n_reps=n_reps, num_devices=N_CORES,
                      unroll=8, wave=False)
        _CACHE[key] = _make_runner(nc)
    return _CACHE[key]


def kernel(**inputs):
    """Full-batch entry point: shard batch 32 across 8 NeuronCores, run the
    Bass program, gather per-core outputs back to [32, 256, 128] fp32."""
    inputs = {k: np.asarray(v) for k, v in inputs.items()}
    consts = prep_consts(inputs)
    in_maps = []
    for c in range(N_CORES):
        m = dict(consts)
        m['xin'] = np.ascontiguousarray(inputs['inputs'][c * 4:(c + 1) * 4], np.float32)
        m['nz'] = np.ascontiguousarray(inputs['noise'][c * 4:(c + 1) * 4], np.float32)
        in_maps.append(m)
    run = _get_runner(1)
    res = run(in_maps)
    out = np.concatenate([res[c]['out'] for c in range(N_CORES)], axis=0)
    return out.astype(np.float32)
